# revision 53
# baseline (speedup 1.0000x reference)
"""Trainium2 Bass kernel for AttentionBlock (softmax over query axis).

Reference computation (per batch b):
    xs   = x[b].reshape(C, S).T                      # (S, C)
    qkv  = xs @ w_proj + b_proj                      # (S, H*3*Dk), head-major
    q,k,v per head h: cols [h*192+0:64 | 64:128 | 128:192]
    sT[j,i] = k_j . q_i * 0.125                      # scores, transposed
    A^T[j,i] = exp(sT[j,i]) / sum_i' exp(sT[j,i'])   # softmax over QUERY axis i
    outT[d,i] = sum_j (v[j,d]/D[j]) * E[j,i]         # normalization folded into v
    res = attn_out @ w_out + b_out + xs              # residual
    out[b] = res.T.reshape(C, F, Hh, W)

Sharding: 8 cores = 4 batches x 2 head-groups (4 heads each).
Each core computes a partial (C, S) output (its 4 heads through w_out);
group-0 cores also add the residual x[b] (group-1 cores receive zeros for
the f32 x input).  Host sums the pair and adds b_out.

Engine balance per core (TimelineSim cost model, ~168.7us total): ScalarE
(exp, 1 elem/cycle/lane @1.2GHz) is the bottleneck at ~134.5us busy; PE
~126us (bf16 matmuls); DVE ~74us (drains + per-j normalization).  All
matmul inputs are bf16 (PE runs 1 cycle/row vs 4 for fp32); the residual
path stays fp32, so the overall relative error is ~5e-4.

Key structural choices:
- Scores are computed transposed (sT[j,i]) so the softmax reduction runs
  along the free axis; exp reads score tiles straight from PSUM and its
  row-sum D comes from a DVE tensor_scalar accum_out pass over E.
- PSUM: 4 banks hold the (128 x 2048) f32 AV accumulator per head-pair,
  4 banks hold two ping-pong (128 x 1024) score tiles.  The exp pipeline
  needs score-slot users per iteration to stay EVEN (ping-pong parity),
  so phase-1 extras are injected in pairs.
- Phase-1 QKV work (v blocks, pair-1 q/k) is drip-fed into pair-0's PE
  slack; dummy warm-up matmuls keep the PE HAM warm through the initial
  DMAs; the output projection reuses the score PSUM slots, its pair-0
  contribution is accumulated during the last attention iteration, and
  the final AV drain runs on the then-idle ScalarE to keep the tail
  DVE-chain (residual adds) short.
"""

import sys

if "/opt/trn_rl_repo" not in sys.path:
    sys.path.insert(0, "/opt/trn_rl_repo")

import numpy as np
import ml_dtypes

import concourse.bass as bass
import concourse.tile as tile
from concourse import bacc, mybir
from concourse.bass_utils import run_bass_kernel_spmd

# problem constants (hardcoded per contract)
N_BATCH = 4
C = 256
S = 2048  # 8*16*16
HEADS = 8
D_K = 64
SCALE = D_K ** -0.5  # 0.125
JB = S // 128  # 16 key blocks
F32 = mybir.dt.float32
BF16 = mybir.dt.bfloat16
AX = mybir.AxisListType
ALU = mybir.AluOpType
AF = mybir.ActivationFunctionType

_NC_CACHE = {}


def build_bass():
    if "nc" in _NC_CACHE:
        return _NC_CACHE["nc"]
    nc = bacc.Bacc("TRN2", target_bir_lowering=False, debug=False, num_devices=8)

    x16_d = nc.dram_tensor("x16", [C, S], BF16, kind="ExternalInput")
    x_d = nc.dram_tensor("x", [C, S], F32, kind="ExternalInput")  # residual (or zeros)
    wqk_d = nc.dram_tensor("wqk", [C, 512], BF16, kind="ExternalInput")
    wv_d = nc.dram_tensor("wv", [C, 256], BF16, kind="ExternalInput")
    bqk_d = nc.dram_tensor("bqk", [128, 4], F32, kind="ExternalInput")
    bv_d = nc.dram_tensor("bv", [1, 512], BF16, kind="ExternalInput")  # [bv|bv]
    wo_d = nc.dram_tensor("wo", [256, C], BF16, kind="ExternalInput")
    out_d = nc.dram_tensor("out", [C, S], F32, kind="ExternalOutput")

    with tile.TileContext(nc) as tc:
        with tc.tile_pool(name="persist", bufs=1) as persist:
            # ---- persistent SBUF tensors ----
            x_b16 = persist.tile([128, 2, S], BF16, tag="x_b16")
            x_f32 = persist.tile([128, 2, S], F32, tag="x_f32")
            wqk_t = persist.tile([128, 2, 512], BF16, tag="wqk")
            wv_t = persist.tile([128, 2, 256], BF16, tag="wv")
            wo_t = persist.tile([128, 2, 256], BF16, tag="wo")
            bqk_t = persist.tile([128, 4], F32, tag="bqk")
            bv_b = persist.tile([128, 512], F32, tag="bv")  # bv replicated x2
            bv_row = persist.tile([1, 512], BF16, tag="bv_row")
            ones_t = persist.tile([1, 128], BF16, tag="ones")
            # qkT rows: [q pair0 | q pair1 | k pair0 | k pair1], each (128, S)
            qkT = persist.tile([128, 4, S], BF16, tag="qkT")
            # v in token-major tiles: v_all[p, it, d] = v[it*128+p, d]
            v_all = persist.tile([128, JB, 256], BF16, tag="v_all")
            attn_all = persist.tile([128, 2, S], BF16, tag="attn")

            # ---- startup loads in exact need-order (DMA engine is serial;
            # HWDGE issue is ~0.6us per dma_start) ----
            x16_r = x16_d.ap().rearrange("(a p) s -> p a s", p=128)
            nc.sync.dma_start(out=wqk_t, in_=wqk_d.ap().rearrange("(a p) s -> p a s", p=128))
            nc.sync.dma_start(out=bqk_t, in_=bqk_d.ap())
            nc.sync.dma_start(out=x_b16[:, :, 0:512], in_=x16_r[:, :, 0:512])
            nc.sync.dma_start(out=x_b16[:, :, 512:1024], in_=x16_r[:, :, 512:1024])
            nc.sync.dma_start(out=x_b16[:, :, 1024:S], in_=x16_r[:, :, 1024:S])
            nc.sync.dma_start(out=wv_t, in_=wv_d.ap().rearrange("(a p) s -> p a s", p=128))
            nc.sync.dma_start(out=bv_row, in_=bv_d.ap())
            nc.vector.memset(ones_t, 1.0)

            # ---- phases 1+2 share one PSUM pool: "av" 4 banks + "sc" 4 banks ----
            with tc.tile_pool(name="p12_psum", bufs=1, space="PSUM") as p12, \
                 tc.tile_pool(name="epool", bufs=3) as epool, \
                 tc.tile_pool(name="small", bufs=3) as small:

                def qk_block(r, ih, width=1024):
                    # qkT[:, r, slice] = wqk[:, r-tile].T @ x (+ bias)
                    nsub = width // 512
                    ps = p12.tile([128, width], F32, name=f"qk{r}_{ih}_{width}", tag="sc", bufs=2)
                    for ct in range(2):
                        for sub in range(nsub):
                            i0 = ih * width + sub * 512
                            nc.tensor.matmul(
                                ps[:, sub * 512:(sub + 1) * 512],
                                wqk_t[:, ct, r * 128:(r + 1) * 128],
                                x_b16[:, ct, i0:i0 + 512],
                                start=(ct == 0), stop=(ct == 1),
                            )
                    nc.vector.tensor_scalar_add(
                        out=qkT[:, r, ih * width:(ih + 1) * width],
                        in0=ps, scalar1=bqk_t[:, r:r + 1],
                    )

                v_flat = v_all.rearrange("p a b -> p (a b)")

                def v_block2(t):
                    # v_all[:, 2t:2t+2, :]: two token blocks in one psum tile
                    ps = p12.tile([128, 512], F32, name=f"v{t}", tag="sc", bufs=2)
                    for sub in range(2):
                        it = 2 * t + sub
                        for ct in range(2):
                            nc.tensor.matmul(
                                ps[:, sub * 256:(sub + 1) * 256],
                                x_b16[:, ct, it * 128:(it + 1) * 128],
                                wv_t[:, ct, :],
                                start=(ct == 0), stop=(ct == 1),
                            )
                    nc.vector.scalar_tensor_tensor(
                        out=v_flat[:, 2 * t * 256:(2 * t + 2) * 256],
                        in0=ps, scalar=1.0, in1=bv_b,
                        op0=ALU.mult, op1=ALU.add,
                    )

                # PE warmup: dummy matmuls on uninitialized data keep the PE
                # HAM busy during the initial DMA so real matmuls run at
                # 2.4GHz; results land in av0 slices that the real AV
                # accumulation overwrites (start=True).
                av0 = p12.tile([128, S], F32, name="av0", tag="av", bufs=1)
                for w in range(8):
                    nc.tensor.matmul(
                        av0[:, (w % 4) * 512:((w % 4) + 1) * 512],
                        attn_all[:, 0, 0:128],
                        attn_all[:, 0, 0:512],
                        start=True, stop=True,
                    )

                # minimal q/k + first v for the first exps; the rest of
                # phase 1 is drip-fed into pair 0's PE slack in PAIRS of
                # psum-slot users per iteration (odd counts rotate the sc
                # ping-pong parity and stall the exp pipeline).
                qk_block(0, 0, 512)
                qk_block(0, 1, 512)
                qk_block(2, 0, 512)

                def bv_broadcast():
                    # broadcast [bv|bv] row to all partitions via a K=1 matmul
                    bvps = p12.tile([128, 512], F32, name="bvps", tag="sc", bufs=2)
                    nc.tensor.matmul(bvps, ones_t, bv_row, start=True, stop=True)
                    nc.vector.tensor_copy(bv_b, bvps)
                extras = {jb: [] for jb in range(JB)}
                extras[1] = [(v_block2, 1), (qk_block, 2, 1, 512)]
                extras[3] = [(v_block2, 2), (qk_block, 2, 2, 512)]
                extras[5] = [(v_block2, 3), (qk_block, 2, 3, 512)]
                extras[7] = [(v_block2, 4), (qk_block, 1, 0, 512)]
                extras[9] = [(v_block2, 5), (qk_block, 1, 1, 512)]
                extras[11] = [(v_block2, 6), (qk_block, 1, 2, 512)]
                extras[13] = [(v_block2, 7), (qk_block, 1, 3, 512)]
                extras[14] = [(qk_block, 3, 0, 512), (qk_block, 3, 1, 512)]
                extras[15] = [(qk_block, 3, 2, 512), (qk_block, 3, 3, 512)]

                op_tiles = {}

                def op_alloc_and_p0(ct, ihh):
                    # output-projection psum tile; pair-0 contribution can be
                    # accumulated early (attn pair 0 has long been drained)
                    ps = p12.tile([128, 1024], F32, name=f"op{ct}{ihh}", tag="sc", bufs=2)
                    for sub in range(2):
                        i0 = ihh * 1024 + sub * 512
                        nc.tensor.matmul(
                            ps[:, sub * 512:(sub + 1) * 512],
                            wo_t[:, 0, ct * 128:(ct + 1) * 128],
                            attn_all[:, 0, i0:i0 + 512],
                            start=True, stop=False,
                        )
                    op_tiles[(ct, ihh)] = ps
                    return ps

                for p in range(2):
                    if p == 0:
                        av = av0
                    else:
                        av = p12.tile([128, S], F32, name="av1", tag="av", bufs=1)
                        # residual/wo inputs not needed until phase 3; load
                        # them while pair-1 attention runs
                        nc.sync.dma_start(out=x_f32, in_=x_d.ap().rearrange("(a p) s -> p a s", p=128))
                        nc.sync.dma_start(out=wo_t, in_=wo_d.ap().rearrange("(a p) s -> p a s", p=128))
                    es = {}
                    vps = {}

                    def av_chunk(h, half, jj, av=av):
                        # half the AV accumulation for previous j-block jj
                        if jj < 0:
                            return
                        hb = 64 * h
                        for isl in (2 * half, 2 * half + 1):
                            nc.tensor.matmul(
                                av[hb:hb + 64, isl * 512:(isl + 1) * 512],
                                vps[(jj, h)],
                                es[(jj, h)][:, isl * 512:(isl + 1) * 512],
                                start=(jj == 0), stop=(jj == JB - 1),
                                tile_position=(0, hb),
                            )

                    for jb in range(JB + 1):
                        jj = jb - 1
                        if jb < JB:
                            # PE order interleaves score-buffer refills with
                            # AV chunks so ScalarE never waits on a refill
                            for h in range(2):
                                if h == 1 and p == 0:
                                    for fn_args in extras[jb]:
                                        fn_args[0](*fn_args[1:])
                                hb = 64 * h
                                e_t = epool.tile([128, S], BF16, tag=f"e{h}")
                                # first h-block runs 512-wide so the exp chain
                                # starts before the x16 hi-half lands
                                wid = 512 if (p == 0 and jb == 0 and h == 0) else 1024
                                nih = S // wid
                                for ih in range(nih):
                                    sc = p12.tile([128, wid], F32, name=f"sc{jb}{h}{ih}", tag="sc", bufs=2)
                                    for sub in range(wid // 512):
                                        i0 = ih * wid + sub * 512
                                        nc.tensor.matmul(
                                            sc[:, sub * 512:(sub + 1) * 512],
                                            qkT[hb:hb + 64, 2 + p, jb * 128:(jb + 1) * 128],
                                            qkT[hb:hb + 64, p, i0:i0 + 512],
                                            start=True, stop=True,
                                            tile_position=(hb, 0),
                                        )
                                    nc.scalar.activation(
                                        out=e_t[:, ih * wid:(ih + 1) * wid],
                                        in_=sc, func=AF.Exp, scale=float(SCALE),
                                    )
                                    if ih == nih - 1:
                                        # D = sum_i E (free DVE accum on x1.0
                                        # passes; split for finer interleave)
                                        dsum = small.tile([128, 2], F32, tag="dsum")
                                        for dh in range(2):
                                            dsl = slice(dh * 1024, (dh + 1) * 1024)
                                            nc.vector.tensor_scalar(
                                                out=e_t[:, dsl], in0=e_t[:, dsl],
                                                scalar1=1.0, scalar2=0.0,
                                                op0=ALU.mult, op1=ALU.add,
                                                accum_out=dsum[:, dh:dh + 1],
                                            )
                                        dtot = small.tile([128, 1], F32, tag="dtot")
                                        nc.vector.tensor_add(dtot, dsum[:, 0:1], dsum[:, 1:2])
                                        dinv = small.tile([128, 1], F32, tag="dinv")
                                        nc.vector.reciprocal(dinv, dtot)
                                        vp = small.tile([128, 64], BF16, tag=f"vp{h}")
                                        nc.vector.tensor_scalar_mul(
                                            vp, v_all[:, jb, p * 128 + hb:p * 128 + hb + 64], dinv)
                                        es[(jb, h)] = e_t
                                        vps[(jb, h)] = vp
                                    if ih >= nih - 2:
                                        av_chunk(h, ih - (nih - 2), jj)
                                    if p == 0 and jb == 0 and h == 0:
                                        # startup stragglers: hi-half q/k as
                                        # their DMA lands, then bv + v0
                                        if ih == 1:
                                            qk_block(0, 2, 512)
                                            qk_block(0, 3, 512)
                                        elif ih == 2:
                                            bv_broadcast()
                                            v_block2(0)
                        else:
                            if p == 1:
                                # fill the PE gap (AV jj15 waits on the DVE
                                # chain) with the first out-proj p0 matmuls
                                op_alloc_and_p0(0, 0)
                                op_alloc_and_p0(0, 1)
                            for h in range(2):
                                for half in range(2):
                                    av_chunk(h, half, jj)
                    for q4 in range(4):
                        sl = slice(q4 * 512, (q4 + 1) * 512)
                        if p == 1:
                            # ScalarE is idle after the last exp; freeing DVE
                            # for the output adds shortens the tail
                            nc.scalar.activation(out=attn_all[:, p, sl], in_=av[:, sl], func=AF.Copy)
                        else:
                            nc.vector.tensor_copy(attn_all[:, p, sl], av[:, sl])



                # ---- phase 3: output projection + residual (reuses the
                # sc psum slots, which free up right after the last exps) ----
                with tc.tile_pool(name="outp", bufs=4) as outp:
                    for ct in range(2):
                        for ihh in range(2):
                            ps = op_tiles.get((ct, ihh))
                            if ps is None:
                                ps = op_alloc_and_p0(ct, ihh)
                            for sub in range(2):
                                i0 = ihh * 1024 + sub * 512
                                nc.tensor.matmul(
                                    ps[:, sub * 512:(sub + 1) * 512],
                                    wo_t[:, 1, ct * 128:(ct + 1) * 128],
                                    attn_all[:, 1, i0:i0 + 512],
                                    start=False, stop=True,
                                )
                            outsb = outp.tile([128, 1024], F32, tag="outsb")
                            for q2 in range(2):
                                i0 = ihh * 1024 + q2 * 512
                                nc.vector.tensor_add(
                                    outsb[:, q2 * 512:(q2 + 1) * 512],
                                    x_f32[:, ct, i0:i0 + 512], ps[:, q2 * 512:(q2 + 1) * 512])
                                nc.sync.dma_start(
                                    out=out_d.ap()[ct * 128:(ct + 1) * 128, i0:i0 + 512],
                                    in_=outsb[:, q2 * 512:(q2 + 1) * 512])

    nc.compile()
    _NC_CACHE["nc"] = nc
    return nc


def make_in_maps(x, w_proj, b_proj, w_out):
    """Per-core input dicts. Core c: batch = c % 4, head-group g = c // 4."""
    bf16 = ml_dtypes.bfloat16
    zeros_f32 = np.zeros((C, S), np.float32)
    x16 = [np.ascontiguousarray(x[b].reshape(C, S)).astype(bf16) for b in range(N_BATCH)]
    in_maps = []
    for core in range(8):
        b = core % 4
        g = core // 4
        base = g * 768
        q_cols = [w_proj[:, base + h * 192: base + h * 192 + 64] for h in range(4)]
        k_cols = [w_proj[:, base + h * 192 + 64: base + h * 192 + 128] for h in range(4)]
        v_cols = [w_proj[:, base + h * 192 + 128: base + h * 192 + 192] for h in range(4)]
        bq = [b_proj[base + h * 192: base + h * 192 + 64] for h in range(4)]
        bk = [b_proj[base + h * 192 + 64: base + h * 192 + 128] for h in range(4)]
        bv = [b_proj[base + h * 192 + 128: base + h * 192 + 192] for h in range(4)]
        wqk = np.concatenate(q_cols + k_cols, axis=1)  # (256, 512)
        wv = np.concatenate(v_cols, axis=1)  # (256, 256)
        bqk = np.concatenate(bq + bk)  # (512,)
        bvv = np.concatenate(bv)  # (256,)
        in_maps.append({
            "x16": x16[b],
            "x": np.ascontiguousarray(x[b].reshape(C, S), dtype=np.float32) if g == 0 else zeros_f32,
            "wqk": np.ascontiguousarray(wqk).astype(bf16),
            "wv": np.ascontiguousarray(wv).astype(bf16),
            "bqk": np.ascontiguousarray(bqk.reshape(4, 128).T),
            "bv": np.ascontiguousarray(np.concatenate([bvv, bvv])[None, :]).astype(bf16),
            "wo": np.ascontiguousarray(w_out[g * 256:(g + 1) * 256, :]).astype(bf16),
        })
    return in_maps


def kernel(x, w_proj, b_proj, w_out, b_out):
    x = np.asarray(x, dtype=np.float32)
    w_proj = np.asarray(w_proj, dtype=np.float32)
    b_proj = np.asarray(b_proj, dtype=np.float32)
    w_out = np.asarray(w_out, dtype=np.float32)
    b_out = np.asarray(b_out, dtype=np.float32)
    nc = build_bass()
    in_maps = make_in_maps(x, w_proj, b_proj, w_out)
    try:
        res = run_bass_kernel_spmd(nc, in_maps, core_ids=list(range(8)))
    except Exception:
        # transient NRT/tunnel failures happen occasionally; retry once
        res = run_bass_kernel_spmd(nc, in_maps, core_ids=list(range(8)))
    out = np.empty((N_BATCH, C, S), np.float32)
    for b in range(N_BATCH):
        out[b] = res.results[b]["out"] + res.results[b + 4]["out"]
    out += b_out[None, :, None]
    return out.reshape(x.shape)


# revision 60
# speedup vs baseline: 1.0272x; 1.0272x over previous
"""Trainium2 Bass kernel for AttentionBlock (softmax over query axis).

Reference computation (per batch b):
    xs   = x[b].reshape(C, S).T                      # (S, C)
    qkv  = xs @ w_proj + b_proj                      # (S, H*3*Dk), head-major
    q,k,v per head h: cols [h*192+0:64 | 64:128 | 128:192]
    sT[j,i] = k_j . q_i * 0.125                      # scores, transposed
    A^T[j,i] = exp(sT[j,i]) / sum_i' exp(sT[j,i'])   # softmax over QUERY axis i
    outT[d,i] = sum_j (v[j,d]/D[j]) * E[j,i]         # normalization folded into v
    res = attn_out @ w_out + b_out + xs              # residual
    out[b] = res.T.reshape(C, F, Hh, W)

Sharding: 8 cores = 4 batches x 2 head-groups (4 heads each).
Each core computes a partial (C, S) output (its 4 heads through w_out);
group-0 cores also add the residual x[b] (group-1 cores receive zeros for
the f32 x input).  Host sums the pair and adds b_out.

Engine balance per core (TimelineSim cost model, ~168.7us total): ScalarE
(exp, 1 elem/cycle/lane @1.2GHz) is the bottleneck at ~134.5us busy; PE
~126us (bf16 matmuls); DVE ~74us (drains + per-j normalization).  All
matmul inputs are bf16 (PE runs 1 cycle/row vs 4 for fp32); the residual
path stays fp32, so the overall relative error is ~5e-4.

Key structural choices:
- Scores are computed transposed (sT[j,i]) so the softmax reduction runs
  along the free axis; exp reads score tiles straight from PSUM and its
  row-sum D comes from a DVE tensor_scalar accum_out pass over E.
- PSUM: 4 banks hold the (128 x 2048) f32 AV accumulator per head-pair,
  4 banks hold two ping-pong (128 x 1024) score tiles.  The exp pipeline
  needs score-slot users per iteration to stay EVEN (ping-pong parity),
  so phase-1 extras are injected in pairs.
- Phase-1 QKV work (v blocks, pair-1 q/k) is drip-fed into pair-0's PE
  slack; dummy warm-up matmuls keep the PE HAM warm through the initial
  DMAs; the output projection reuses the score PSUM slots, its pair-0
  contribution is accumulated during the last attention iteration, and
  the final AV drain runs on the then-idle ScalarE to keep the tail
  DVE-chain (residual adds) short.
"""

import sys

if "/opt/trn_rl_repo" not in sys.path:
    sys.path.insert(0, "/opt/trn_rl_repo")

import numpy as np
import ml_dtypes

import concourse.bass as bass
import concourse.tile as tile
from concourse import bacc, mybir
from concourse.bass_utils import run_bass_kernel_spmd

# problem constants (hardcoded per contract)
N_BATCH = 4
C = 256
S = 2048  # 8*16*16
HEADS = 8
D_K = 64
SCALE = D_K ** -0.5  # 0.125
JB = S // 128  # 16 key blocks
F32 = mybir.dt.float32
BF16 = mybir.dt.bfloat16
AX = mybir.AxisListType
ALU = mybir.AluOpType
AF = mybir.ActivationFunctionType

_NC_CACHE = {}


def build_bass():
    if "nc" in _NC_CACHE:
        return _NC_CACHE["nc"]
    nc = bacc.Bacc("TRN2", target_bir_lowering=False, debug=False, num_devices=8)

    x16_d = nc.dram_tensor("x16", [C, S], BF16, kind="ExternalInput")
    x_d = nc.dram_tensor("x", [C, S], F32, kind="ExternalInput")  # residual (or zeros)
    wqk_d = nc.dram_tensor("wqk", [C, 512], BF16, kind="ExternalInput")
    wv_d = nc.dram_tensor("wv", [C, 256], BF16, kind="ExternalInput")
    bqk_d = nc.dram_tensor("bqk", [128, 4], F32, kind="ExternalInput")
    bv_d = nc.dram_tensor("bv", [1, 512], BF16, kind="ExternalInput")  # [bv|bv]
    wo_d = nc.dram_tensor("wo", [256, C], BF16, kind="ExternalInput")
    out_d = nc.dram_tensor("out", [C, S], F32, kind="ExternalOutput")

    with tile.TileContext(nc) as tc:
        with tc.tile_pool(name="persist", bufs=1) as persist:
            # ---- persistent SBUF tensors ----
            x_b16 = persist.tile([128, 2, S], BF16, tag="x_b16")
            x_f32 = persist.tile([128, 2, S], F32, tag="x_f32")
            wqk_t = persist.tile([128, 2, 512], BF16, tag="wqk")
            wv_t = persist.tile([128, 2, 256], BF16, tag="wv")
            wo_t = persist.tile([128, 2, 256], BF16, tag="wo")
            bqk_t = persist.tile([128, 4], F32, tag="bqk")
            bv_b = persist.tile([128, 512], F32, tag="bv")  # bv replicated x2
            bv_row = persist.tile([1, 512], BF16, tag="bv_row")
            ones_t = persist.tile([1, 128], BF16, tag="ones")
            # qkT rows: [q pair0 | q pair1 | k pair0 | k pair1], each (128, S)
            qkT = persist.tile([128, 4, S], BF16, tag="qkT")
            # v in token-major tiles: v_all[p, it, d] = v[it*128+p, d]
            v_all = persist.tile([128, JB, 256], BF16, tag="v_all")
            attn_all = persist.tile([128, 2, S], BF16, tag="attn")

            # ---- startup loads in exact need-order (DMA engine is serial;
            # HWDGE issue is ~0.6us per dma_start) ----
            x16_r = x16_d.ap().rearrange("(a p) s -> p a s", p=128)
            nc.sync.dma_start(out=wqk_t, in_=wqk_d.ap().rearrange("(a p) s -> p a s", p=128))
            nc.sync.dma_start(out=bqk_t, in_=bqk_d.ap())
            nc.sync.dma_start(out=x_b16[:, :, 0:512], in_=x16_r[:, :, 0:512])
            nc.sync.dma_start(out=x_b16[:, :, 512:1024], in_=x16_r[:, :, 512:1024])
            nc.sync.dma_start(out=x_b16[:, :, 1024:S], in_=x16_r[:, :, 1024:S])
            nc.sync.dma_start(out=wv_t, in_=wv_d.ap().rearrange("(a p) s -> p a s", p=128))
            nc.sync.dma_start(out=bv_row, in_=bv_d.ap())
            nc.vector.memset(ones_t, 1.0)

            # ---- phases 1+2 share one PSUM pool: "av" 4 banks + "sc" 4 banks ----
            with tc.tile_pool(name="p12_psum", bufs=1, space="PSUM") as p12, \
                 tc.tile_pool(name="epool", bufs=3) as epool, \
                 tc.tile_pool(name="small", bufs=3) as small:

                def qk_block(r, ih, width=1024, ps=None):
                    # qkT[:, r, slice] = wqk[:, r-tile].T @ x (+ bias)
                    nsub = width // 512
                    if ps is None:
                        ps = p12.tile([128, width], F32, name=f"qk{r}_{ih}_{width}", tag="sc", bufs=2)
                    for ct in range(2):
                        for sub in range(nsub):
                            i0 = ih * width + sub * 512
                            nc.tensor.matmul(
                                ps[:, sub * 512:(sub + 1) * 512],
                                wqk_t[:, ct, r * 128:(r + 1) * 128],
                                x_b16[:, ct, i0:i0 + 512],
                                start=(ct == 0), stop=(ct == 1),
                            )
                    nc.vector.tensor_scalar_add(
                        out=qkT[:, r, ih * width:(ih + 1) * width],
                        in0=ps, scalar1=bqk_t[:, r:r + 1],
                    )

                v_flat = v_all.rearrange("p a b -> p (a b)")

                def v_block2(t, ps=None):
                    # v_all[:, 2t:2t+2, :]: two token blocks in one psum tile
                    if ps is None:
                        ps = p12.tile([128, 512], F32, name=f"v{t}", tag="sc", bufs=2)
                    for sub in range(2):
                        it = 2 * t + sub
                        for ct in range(2):
                            nc.tensor.matmul(
                                ps[:, sub * 256:(sub + 1) * 256],
                                x_b16[:, ct, it * 128:(it + 1) * 128],
                                wv_t[:, ct, :],
                                start=(ct == 0), stop=(ct == 1),
                            )
                    nc.vector.scalar_tensor_tensor(
                        out=v_flat[:, 2 * t * 256:(2 * t + 2) * 256],
                        in0=ps, scalar=1.0, in1=bv_b,
                        op0=ALU.mult, op1=ALU.add,
                    )

                # PE warmup: dummy matmuls on uninitialized data keep the PE
                # HAM busy during the initial DMA so real matmuls run at
                # 2.4GHz; results land in av0 slices that the real AV
                # accumulation overwrites (start=True).
                av0 = p12.tile([128, S], F32, name="av0", tag="av", bufs=1)
                for w in range(8):
                    nc.tensor.matmul(
                        av0[:, (w % 4) * 512:((w % 4) + 1) * 512],
                        attn_all[:, 0, 0:128],
                        attn_all[:, 0, 0:512],
                        start=True, stop=True,
                    )

                # minimal q/k + first v for the first exps; the rest of
                # phase 1 is drip-fed into pair 0's PE slack in PAIRS of
                # psum-slot users per iteration (odd counts rotate the sc
                # ping-pong parity and stall the exp pipeline).
                qk_block(0, 0, 512)
                qk_block(0, 1, 512)
                qk_block(2, 0, 512)

                def bv_broadcast(ps=None):
                    # broadcast [bv|bv] row to all partitions via a K=1 matmul
                    if ps is None:
                        ps = p12.tile([128, 512], F32, name="bvps", tag="sc", bufs=2)
                    nc.tensor.matmul(ps, ones_t, bv_row, start=True, stop=True)
                    nc.vector.tensor_copy(bv_b, ps)
                extras = {jb: [] for jb in range(JB)}
                extras_end = {jb: [] for jb in range(JB)}
                # parked in av0 regions (no sc-slot pressure); drains complete
                # well before AV[jj=0] lands in those banks
                # all parked emissions must precede the first AV[0] chunk
                # (emitted early in jb1) or they would clobber the accumulator
                extras_end[0] = [
                    (v_block2, 1, av0[:, 512:1024]),
                    (v_block2, 2, av0[:, 1024:1536]),
                    (v_block2, 3, av0[:, 1536:2048]),
                    (qk_block, 2, 1, 512, av0[:, 0:512]),
                    (qk_block, 2, 2, 512, av0[:, 512:1024]),
                    (qk_block, 2, 3, 512, av0[:, 1024:1536]),
                ]  # region chains stay depth<=3; all drain before AV[0] lands
                extras[5] = [(v_block2, 4), (qk_block, 1, 0, 512)]
                extras[7] = [(v_block2, 5), (qk_block, 1, 1, 512)]
                extras[9] = [(v_block2, 6), (qk_block, 1, 2, 512)]
                extras[11] = [(v_block2, 7), (qk_block, 1, 3, 512)]
                extras[13] = [(qk_block, 3, 0, 512), (qk_block, 3, 1, 512)]
                extras[15] = [(qk_block, 3, 2, 512), (qk_block, 3, 3, 512)]

                op_tiles = {}

                def op_alloc_and_p0(ct, ihh):
                    # output-projection psum tile; pair-0 contribution can be
                    # accumulated early (attn pair 0 has long been drained)
                    ps = p12.tile([128, 1024], F32, name=f"op{ct}{ihh}", tag="sc", bufs=2)
                    for sub in range(2):
                        i0 = ihh * 1024 + sub * 512
                        nc.tensor.matmul(
                            ps[:, sub * 512:(sub + 1) * 512],
                            wo_t[:, 0, ct * 128:(ct + 1) * 128],
                            attn_all[:, 0, i0:i0 + 512],
                            start=True, stop=False,
                        )
                    op_tiles[(ct, ihh)] = ps
                    return ps

                for p in range(2):
                    if p == 0:
                        av = av0
                    else:
                        av = p12.tile([128, S], F32, name="av1", tag="av", bufs=1)
                        # residual/wo inputs not needed until phase 3; load
                        # them while pair-1 attention runs
                        nc.sync.dma_start(out=x_f32, in_=x_d.ap().rearrange("(a p) s -> p a s", p=128))
                        nc.sync.dma_start(out=wo_t, in_=wo_d.ap().rearrange("(a p) s -> p a s", p=128))
                    es = {}
                    vps = {}

                    def av_chunk(h, half, jj, av=av):
                        # half the AV accumulation for previous j-block jj
                        if jj < 0:
                            return
                        hb = 64 * h
                        for isl in (2 * half, 2 * half + 1):
                            nc.tensor.matmul(
                                av[hb:hb + 64, isl * 512:(isl + 1) * 512],
                                vps[(jj, h)],
                                es[(jj, h)][:, isl * 512:(isl + 1) * 512],
                                start=(jj == 0), stop=(jj == JB - 1),
                                tile_position=(0, hb),
                            )

                    # AV runs two j-blocks behind its exp (depth-2 pipeline):
                    # the parked startup drains in av0 then never defer it
                    for jb in range(JB + 2):
                        jj = jb - 2
                        if jb < JB:
                            # PE order interleaves score-buffer refills with
                            # AV chunks so ScalarE never waits on a refill
                            for h in range(2):
                                if h == 1 and p == 0:
                                    for fn_args in extras[jb]:
                                        fn_args[0](*fn_args[1:])
                                hb = 64 * h
                                e_t = epool.tile([128, S], BF16, tag=f"e{h}")
                                # first h-block runs 512-wide so the exp chain
                                # starts before the x16 hi-half lands
                                wid = 512 if (p == 0 and jb == 0 and h == 0) else 1024
                                nih = S // wid
                                for ih in range(nih):
                                    sc = p12.tile([128, wid], F32, name=f"sc{jb}{h}{ih}", tag="sc", bufs=2)
                                    for sub in range(wid // 512):
                                        i0 = ih * wid + sub * 512
                                        nc.tensor.matmul(
                                            sc[:, sub * 512:(sub + 1) * 512],
                                            qkT[hb:hb + 64, 2 + p, jb * 128:(jb + 1) * 128],
                                            qkT[hb:hb + 64, p, i0:i0 + 512],
                                            start=True, stop=True,
                                            tile_position=(hb, 0),
                                        )
                                    nc.scalar.activation(
                                        out=e_t[:, ih * wid:(ih + 1) * wid],
                                        in_=sc, func=AF.Exp, scale=float(SCALE),
                                    )
                                    if ih == nih - 1:
                                        # D = sum_i E (free DVE accum on x1.0
                                        # passes; split for finer interleave)
                                        dsum = small.tile([128, 2], F32, tag="dsum")
                                        for dh in range(2):
                                            dsl = slice(dh * 1024, (dh + 1) * 1024)
                                            nc.vector.tensor_scalar(
                                                out=e_t[:, dsl], in0=e_t[:, dsl],
                                                scalar1=1.0, scalar2=0.0,
                                                op0=ALU.mult, op1=ALU.add,
                                                accum_out=dsum[:, dh:dh + 1],
                                            )
                                        dtot = small.tile([128, 1], F32, tag="dtot")
                                        nc.vector.tensor_add(dtot, dsum[:, 0:1], dsum[:, 1:2])
                                        dinv = small.tile([128, 1], F32, tag="dinv")
                                        nc.vector.reciprocal(dinv, dtot)
                                        vp = small.tile([128, 64], BF16, tag=f"vp{h}")
                                        nc.vector.tensor_scalar_mul(
                                            vp, v_all[:, jb, p * 128 + hb:p * 128 + hb + 64], dinv)
                                        es[(jb, h)] = e_t
                                        vps[(jb, h)] = vp
                                    if ih >= nih - 2:
                                        av_chunk(h, ih - (nih - 2), jj)
                                    if p == 0 and jb == 0 and h == 0:
                                        # startup stragglers: hi-half q/k as
                                        # their DMA lands, then bv + v0
                                        if ih == 1:
                                            # parked in av0's idle banks: no
                                            # score-slot round-trip, so the
                                            # i2/i3 exps chain without gaps
                                            qk_block(0, 2, 512, ps=av0[:, 512:1024])
                                            qk_block(0, 3, 512, ps=av0[:, 1024:1536])
                                        elif ih == 2:
                                            bv_broadcast(ps=av0[:, 1536:2048])
                                            v_block2(0, ps=av0[:, 0:512])
                            if p == 0:
                                for fn_args in extras_end[jb]:
                                    fn_args[0](*fn_args[1:-1], ps=fn_args[-1])
                        else:
                            if p == 1 and jb == JB:
                                # fill the PE gap (AV tail waits on the DVE
                                # chain) with the first out-proj p0 matmuls
                                op_alloc_and_p0(0, 0)
                                op_alloc_and_p0(0, 1)
                            for h in range(2):
                                for half in range(2):
                                    av_chunk(h, half, jj)
                    for q4 in range(4):
                        sl = slice(q4 * 512, (q4 + 1) * 512)
                        if p == 1:
                            # ScalarE is idle after the last exp; freeing DVE
                            # for the output adds shortens the tail
                            nc.scalar.activation(out=attn_all[:, p, sl], in_=av[:, sl], func=AF.Copy)
                        else:
                            nc.vector.tensor_copy(attn_all[:, p, sl], av[:, sl])



                # ---- phase 3: output projection + residual (reuses the
                # sc psum slots, which free up right after the last exps) ----
                with tc.tile_pool(name="outp", bufs=4) as outp:
                    for ct in range(2):
                        for ihh in range(2):
                            ps = op_tiles.get((ct, ihh))
                            if ps is None:
                                ps = op_alloc_and_p0(ct, ihh)
                            for sub in range(2):
                                i0 = ihh * 1024 + sub * 512
                                nc.tensor.matmul(
                                    ps[:, sub * 512:(sub + 1) * 512],
                                    wo_t[:, 1, ct * 128:(ct + 1) * 128],
                                    attn_all[:, 1, i0:i0 + 512],
                                    start=False, stop=True,
                                )
                            outsb = outp.tile([128, 1024], F32, tag="outsb")
                            for q2 in range(2):
                                i0 = ihh * 1024 + q2 * 512
                                nc.vector.tensor_add(
                                    outsb[:, q2 * 512:(q2 + 1) * 512],
                                    x_f32[:, ct, i0:i0 + 512], ps[:, q2 * 512:(q2 + 1) * 512])
                                nc.sync.dma_start(
                                    out=out_d.ap()[ct * 128:(ct + 1) * 128, i0:i0 + 512],
                                    in_=outsb[:, q2 * 512:(q2 + 1) * 512])

    nc.compile()
    _NC_CACHE["nc"] = nc
    return nc


def make_in_maps(x, w_proj, b_proj, w_out):
    """Per-core input dicts. Core c: batch = c % 4, head-group g = c // 4."""
    bf16 = ml_dtypes.bfloat16
    zeros_f32 = np.zeros((C, S), np.float32)
    x16 = [np.ascontiguousarray(x[b].reshape(C, S)).astype(bf16) for b in range(N_BATCH)]
    in_maps = []
    for core in range(8):
        b = core % 4
        g = core // 4
        base = g * 768
        q_cols = [w_proj[:, base + h * 192: base + h * 192 + 64] for h in range(4)]
        k_cols = [w_proj[:, base + h * 192 + 64: base + h * 192 + 128] for h in range(4)]
        v_cols = [w_proj[:, base + h * 192 + 128: base + h * 192 + 192] for h in range(4)]
        bq = [b_proj[base + h * 192: base + h * 192 + 64] for h in range(4)]
        bk = [b_proj[base + h * 192 + 64: base + h * 192 + 128] for h in range(4)]
        bv = [b_proj[base + h * 192 + 128: base + h * 192 + 192] for h in range(4)]
        wqk = np.concatenate(q_cols + k_cols, axis=1)  # (256, 512)
        wv = np.concatenate(v_cols, axis=1)  # (256, 256)
        bqk = np.concatenate(bq + bk)  # (512,)
        bvv = np.concatenate(bv)  # (256,)
        in_maps.append({
            "x16": x16[b],
            "x": np.ascontiguousarray(x[b].reshape(C, S), dtype=np.float32) if g == 0 else zeros_f32,
            "wqk": np.ascontiguousarray(wqk).astype(bf16),
            "wv": np.ascontiguousarray(wv).astype(bf16),
            "bqk": np.ascontiguousarray(bqk.reshape(4, 128).T),
            "bv": np.ascontiguousarray(np.concatenate([bvv, bvv])[None, :]).astype(bf16),
            "wo": np.ascontiguousarray(w_out[g * 256:(g + 1) * 256, :]).astype(bf16),
        })
    return in_maps


def kernel(x, w_proj, b_proj, w_out, b_out):
    x = np.asarray(x, dtype=np.float32)
    w_proj = np.asarray(w_proj, dtype=np.float32)
    b_proj = np.asarray(b_proj, dtype=np.float32)
    w_out = np.asarray(w_out, dtype=np.float32)
    b_out = np.asarray(b_out, dtype=np.float32)
    nc = build_bass()
    in_maps = make_in_maps(x, w_proj, b_proj, w_out)
    try:
        res = run_bass_kernel_spmd(nc, in_maps, core_ids=list(range(8)))
    except Exception:
        # transient NRT/tunnel failures happen occasionally; retry once
        res = run_bass_kernel_spmd(nc, in_maps, core_ids=list(range(8)))
    out = np.empty((N_BATCH, C, S), np.float32)
    for b in range(N_BATCH):
        out[b] = res.results[b]["out"] + res.results[b + 4]["out"]
    out += b_out[None, :, None]
    return out.reshape(x.shape)
